# revision 1
# baseline (speedup 1.0000x reference)
"""Trainium2 Bass kernel for nn_BiLSTMNet (2-layer BiLSTM + pair-gather MLP).

Strategy: fully data-parallel across 8 cores (16 sentences each), both LSTM
directions fused per core via block-diagonal matmuls.  Input projections are
computed just-in-time into PSUM (128-slot chunks); the recurrent matmul
accumulates on top (start=False), so gate pre-activations never touch DVE.
h^T is produced by DMA-transpose (bf16) and stored to DRAM in both processing
and reverse order so layer-1 / MLP consumers always read ascending columns.
MLP is decomposed as U0 = h1 @ w1[:, :2H].T, U1 = h1 @ w1[:, 2H:].T computed
for all (t, b), then the conf-pair gather is a row gather + add + tanh.
"""
import sys
sys.path.insert(0, "/opt/trn_rl_repo")
import numpy as np
import ml_dtypes

import concourse.bass as bass
import concourse.tile as tile
from concourse import mybir, bacc
from concourse.bass_utils import run_bass_kernel_spmd

BF16 = mybir.dt.bfloat16
F32 = mybir.dt.float32
I32 = mybir.dt.int32
AF = mybir.ActivationFunctionType
ALU = mybir.AluOpType

V, E, H, B, C = 32000, 200, 200, 128, 256
T_FULL = 512
BL = 16            # sentences per core
NCORE = 8
EP = 256           # padded E (dma-transpose wants 128-col blocks)
HP = 256           # padded H
G4 = 800           # 4*H gate width
CHT = 4            # timesteps per xg chunk (chunk = CHT*2*BL = 128 slots)


def build(T, n_cores, NPT):
    NCH = T // CHT
    NSLOT = T * BL                # per-direction (t,b) slots
    NUC = NSLOT // 128            # U-phase chunks

    nc = bacc.Bacc("TRN2", target_bir_lowering=False, debug=False,
                   enable_asserts=True, num_devices=n_cores)

    def din(name, shape, dt):
        return nc.dram_tensor(name, shape, dt, kind="ExternalInput").ap()

    def dout(name, shape, dt):
        return nc.dram_tensor(name, shape, dt, kind="ExternalOutput").ap()

    emb = din("emb", [V, E], BF16)
    W0s = din("W0s", [2 * EP, G4], BF16)      # xg0 stream (block-diag K rows)
    Whh0s = din("Whh0s", [2 * HP, G4], BF16)  # L0 recurrent stream
    W1sf = din("W1sf", [2 * H + 1, G4], BF16)  # xg1 stream, fwd block
    W1sb = din("W1sb", [2 * H + 1, G4], BF16)  # xg1 stream, bwd block
    Whh1s = din("Whh1s", [2 * HP, G4], BF16)
    WU = din("WU", [2 * H + 1, G4], BF16)      # U stream [w1a.T | w1b.T] + bw1 row
    W2s = din("W2s", [4 * 128, 4], BF16)       # w2.T padded to 512 rows + bw2 at 511
    tokf = din("tokf", [CHT * BL, NCH], I32)   # [slot, chunk]
    tokb = din("tokb", [CHT * BL, NCH], I32)
    uidx0 = din("uidx0", [128, NPT], I32)
    uidx1 = din("uidx1", [128, NPT], I32)
    umask0 = din("umask0", [128, NPT], F32)
    umask1 = din("umask1", [128, NPT], F32)
    bw1m = din("bw1m", [128, 2 * H], F32)

    OUT = dout("OUT", [NPT * 128, 4], F32)

    # internal DRAM
    # h0T rows: [0:200] f-proc | [200:400] b-rev | [400] ones |
    #           [401:601] f-rev | [601:801] b-proc | [801] ones
    h0T = nc.dram_tensor("h0T", [802, NSLOT], BF16).ap()
    # h1T rows: [0:200] f-proc | [200:400] b-rev | [400] ones
    h1T = nc.dram_tensor("h1T", [401, NSLOT], BF16).ap()
    U0 = nc.dram_tensor("U0", [NSLOT, 2 * H], F32).ap()
    U1 = nc.dram_tensor("U1", [NSLOT, 2 * H], F32).ap()

    with tile.TileContext(nc) as tc:
        with tc.tile_pool(name="const", bufs=1) as cp, \
             tc.tile_pool(name="state", bufs=1) as sp:

            # ---- load weight streams into SBUF K-chunk tiles
            def load_stream(src, nrows, ncols):
                tiles = []
                r = 0
                while r < nrows:
                    h_ = min(128, nrows - r)
                    t_ = cp.tile([h_, ncols], BF16, tag=f"st{src.name}{r}", name=f"st{src.name}{r}")
                    nc.sync.dma_start(out=t_[:], in_=src[r:r + h_, :])
                    tiles.append(t_)
                    r += h_
                return tiles

            W0t = load_stream(W0s.tensor.ap(), 2 * EP, G4)      # 4 x [128, 800]
            Whh0t = load_stream(Whh0s.tensor.ap(), 2 * HP, G4)  # 4
            W1ft = load_stream(W1sf.tensor.ap(), 2 * H + 1, G4)  # [128,128,128,17]
            W1bt = load_stream(W1sb.tensor.ap(), 2 * H + 1, G4)
            Whh1t = load_stream(Whh1s.tensor.ap(), 2 * HP, G4)
            WUt = load_stream(WU.tensor.ap(), 2 * H + 1, G4)
            W2t = load_stream(W2s.tensor.ap(), 4 * 128, 4)       # 4 x [128, 4]

            # token index tiles (slot-major: [64, NCH])
            tokf_t = cp.tile([CHT * BL, NCH], I32)
            tokb_t = cp.tile([CHT * BL, NCH], I32)
            nc.sync.dma_start(out=tokf_t[:], in_=tokf[:])
            nc.sync.dma_start(out=tokb_t[:], in_=tokb[:])

            # ones rows in h0T/h1T (bias rows consumed via lhsT chunk DMAs)
            ones_row = cp.tile([1, NSLOT], BF16)
            nc.vector.memset(ones_row[:], 1.0)
            nc.sync.dma_start(out=h0T[400:401, :], in_=ones_row[:])
            nc.sync.dma_start(out=h0T[801:802, :], in_=ones_row[:])
            nc.sync.dma_start(out=h1T[400:401, :], in_=ones_row[:])

            # ---- persistent state tiles
            # x gather tiles (per chunk parity): cols 200:255 zero, col 255 one
            xf = [sp.tile([CHT * BL, EP], BF16, tag=f"xf{i}", name=f"xf{i}") for i in range(2)]
            xb = [sp.tile([CHT * BL, EP], BF16, tag=f"xb{i}", name=f"xb{i}") for i in range(2)]
            for t_ in xf + xb:
                nc.vector.memset(t_[:], 0.0)
                nc.vector.memset(t_[:, EP - 1:EP], 1.0)
            # xg lhsT tiles (block-diag): C0..C3 per parity
            Ct = [[sp.tile([128, 128], BF16, tag=f"C{i}{j}", name=f"C{i}{j}") for i in range(4)]
                  for j in range(2)]
            # rec lhsT tiles A0..A3 per step parity
            At = [[sp.tile([128, 2 * BL], BF16, tag=f"A{i}{j}", name=f"A{i}{j}") for i in range(4)]
                  for j in range(2)]
            # L1 xg lhsT tiles D0..D7 per parity (last of each block is 17 rows)
            Dt = [[sp.tile([17 if i in (3, 7) else 128, 128], BF16, tag=f"D{i}{j}", name=f"D{i}{j}")
                   for i in range(8)] for j in range(2)]
            for j in range(2):
                for t_ in Ct[j] + At[j] + Dt[j]:
                    nc.vector.memset(t_[:], 0.0)
            # LSTM state: S = [c | tg] fp32; h per parity
            S = sp.tile([2 * BL, 2 * H], F32)
            ht = [sp.tile([2 * BL, HP], BF16, tag=f"h{i}", name=f"h{i}") for i in range(2)]
            for t_ in ht:
                nc.vector.memset(t_[:], 0.0)
            # identity for PE transposes
            ident32 = sp.tile([32, 32], BF16, name="ident32")
            from concourse.masks import make_identity
            make_identity(nc, ident32[:])

            NB = 2 * BL  # 32 rows per step (f+b)

            with tc.tile_pool(name="work", bufs=2) as wp, \
                 tc.tile_pool(name="xps", bufs=2, space="PSUM") as xps, \
                 tc.tile_pool(name="tps", bufs=1, space="PSUM") as tps:

                xg_tiles = {}

                def emit_xg0_chunk(k):
                    par = k % 2
                    gxf = xf[par]
                    gxb = xb[par]
                    nc.gpsimd.indirect_dma_start(
                        out=gxf[:, 0:E], out_offset=None, in_=emb[:],
                        in_offset=bass.IndirectOffsetOnAxis(ap=tokf_t[:, k:k + 1], axis=0))
                    nc.gpsimd.indirect_dma_start(
                        out=gxb[:, 0:E], out_offset=None, in_=emb[:],
                        in_offset=bass.IndirectOffsetOnAxis(ap=tokb_t[:, k:k + 1], axis=0))
                    # transpose x -> staging, then strided copy into C tiles
                    for i, (src, coff) in enumerate([(gxf, 0), (gxb, BL)]):
                        for half in range(2):
                            stg = wp.tile([128, CHT * BL], BF16, tag="xstg", name="xstg")
                            nc.sync.dma_start_transpose(
                                out=stg[:], in_=src[:, half * 128:half * 128 + 128])
                            ctile = Ct[par][2 * i + half]
                            dst = ctile[:].rearrange("p (a b) -> p a b", b=NB)[:, :, coff:coff + BL]
                            s3 = stg[:].rearrange("p (a b) -> p a b", b=BL)
                            nc.vector.tensor_copy(dst, s3)
                    xgf = xps.tile([128, 400], F32, space="PSUM", tag="xgf", name="xgf", padded_shape=[128, 512])
                    xgg = xps.tile([128, 200], F32, space="PSUM", tag="xgg", name="xgg", padded_shape=[128, 512])
                    xgo = xps.tile([128, 200], F32, space="PSUM", tag="xgo", name="xgo", padded_shape=[128, 512])
                    xg_tiles[("L0", k)] = (xgf, xgg, xgo)
                    for kc in range(4):
                        for (t_, n0, n1) in ((xgf, 0, 400), (xgg, 400, 600), (xgo, 600, G4)):
                            nc.tensor.matmul(t_[:, 0:n1 - n0], Ct[par][kc][:],
                                             W0t[kc][:, n0:n1],
                                             start=(kc == 0), stop=(kc == 3))

                def emit_xg1_chunk(k):
                    par = k % 2
                    c0 = k * CHT * BL
                    cw = CHT * BL
                    rowsets = [(0, 128), (128, 256), (256, 384), (384, 401),
                               (401, 529), (529, 657), (657, 785), (785, 802)]
                    for i, (r0, r1) in enumerate(rowsets):
                        dtile = Dt[par][i]
                        coff = 0 if i < 4 else BL
                        dst = dtile[:].rearrange("p (a b) -> p a b", b=NB)[:, :, coff:coff + BL]
                        src = h0T[r0:r1, c0:c0 + cw].rearrange("p (a b) -> p a b", b=BL)
                        nc.sync.dma_start(out=dst, in_=src)
                    xgf = xps.tile([128, 400], F32, space="PSUM", tag="xgf", name="xgf", padded_shape=[128, 512])
                    xgg = xps.tile([128, 200], F32, space="PSUM", tag="xgg", name="xgg", padded_shape=[128, 512])
                    xgo = xps.tile([128, 200], F32, space="PSUM", tag="xgo", name="xgo", padded_shape=[128, 512])
                    xg_tiles[("L1", k)] = (xgf, xgg, xgo)
                    streams = [W1ft[0], W1ft[1], W1ft[2], W1ft[3],
                               W1bt[0], W1bt[1], W1bt[2], W1bt[3]]
                    for kc in range(8):
                        for (t_, n0, n1) in ((xgf, 0, 400), (xgg, 400, 600), (xgo, 600, G4)):
                            nc.tensor.matmul(t_[:, 0:n1 - n0], Dt[par][kc][:],
                                             streams[kc][:, n0:n1],
                                             start=(kc == 0), stop=(kc == 7))

                def emit_step(layer, p, T_, Whht, store_all):
                    par = p % 2
                    k = p // CHT
                    r = (p % CHT) * NB
                    xgf, xgg, xgo = xg_tiles[(layer, k)]
                    # recurrent matmul: (f,i) tile first, then (g), then (o) so
                    # each activation gates on only its own 4 accumulating MMs
                    for (t_, n0, n1) in ((xgf, 0, 400), (xgg, 400, 600), (xgo, 600, G4)):
                        for kc in range(4):
                            nc.tensor.matmul(t_[r:r + NB, 0:n1 - n0],
                                             At[(p + 1) % 2][kc][:],
                                             Whht[kc][:, n0:n1],
                                             start=False, stop=(kc == 3),
                                             skip_group_check=True,
                                             tile_position=(0, r))
                    # gate nonlinearities (gate order f,i,g,o)
                    sigs = wp.tile([NB, 600], F32, tag="sigs", name="sigs")
                    nc.scalar.activation(sigs[:, 0:2 * H], xgf[r:r + NB, 0:2 * H],
                                         AF.Sigmoid)
                    nc.scalar.activation(S[:, H:2 * H], xgg[r:r + NB, 0:200], AF.Tanh)
                    prod = wp.tile([NB, 2 * H], F32, tag="prod", name="prod")
                    nc.vector.tensor_mul(prod[:], sigs[:, 0:2 * H], S[:, 0:2 * H])
                    nc.vector.tensor_add(S[:, 0:H], prod[:, 0:H], prod[:, H:2 * H])
                    # sigma(o) off the c-critical path
                    nc.scalar.activation(sigs[:, 2 * H:600], xgo[r:r + NB, 0:200],
                                         AF.Sigmoid)
                    tct = wp.tile([NB, H], F32, tag="tct", name="tct")
                    nc.scalar.activation(tct[:], S[:, 0:H], AF.Tanh)
                    hcur = ht[par]
                    nc.vector.tensor_mul(hcur[:, 0:H], sigs[:, 400:600], tct[:])
                    # transpose h via PE -> PSUM, copy slices to next-step lhsT
                    ps1 = tps.tile([128, NB], BF16, space="PSUM", tag="ps1", name="ps1")
                    ps2 = tps.tile([72, NB], BF16, space="PSUM", tag="ps2", name="ps2")
                    nc.tensor.transpose(ps1[:], hcur[:, 0:128], ident32[:])
                    nc.tensor.transpose(ps2[:], hcur[:, 128:200], ident32[:])
                    nA = At[par]
                    nc.vector.tensor_copy(nA[0][:, 0:BL], ps1[:, 0:BL])
                    nc.scalar.copy(nA[1][0:72, 0:BL], ps2[:, 0:BL])
                    nc.vector.tensor_copy(nA[2][:, BL:NB], ps1[:, BL:NB])
                    nc.scalar.copy(nA[3][0:72, BL:NB], ps2[:, BL:NB])
                    # h^T stores from the A tiles (SBUF), off the critical path
                    hT = h0T if layer == "L0" else h1T
                    cp_ = p * BL
                    cr = (T_ - 1 - p) * BL
                    # f-proc rows 0:200 at processing col
                    nc.sync.dma_start(out=hT[0:128, cp_:cp_ + BL], in_=nA[0][:, 0:BL])
                    nc.sync.dma_start(out=hT[128:200, cp_:cp_ + BL], in_=nA[1][0:72, 0:BL])
                    # b-rev rows 200:400 at reversed col
                    nc.sync.dma_start(out=hT[200:328, cr:cr + BL], in_=nA[2][:, BL:NB])
                    nc.sync.dma_start(out=hT[328:400, cr:cr + BL], in_=nA[3][0:72, BL:NB])
                    if store_all:
                        # f-rev rows 401:601, b-proc rows 601:801
                        nc.sync.dma_start(out=hT[401:529, cr:cr + BL], in_=nA[0][:, 0:BL])
                        nc.sync.dma_start(out=hT[529:601, cr:cr + BL], in_=nA[1][0:72, 0:BL])
                        nc.sync.dma_start(out=hT[601:729, cp_:cp_ + BL], in_=nA[2][:, BL:NB])
                        nc.sync.dma_start(out=hT[729:801, cp_:cp_ + BL], in_=nA[3][0:72, BL:NB])

                def reset_states():
                    nc.vector.memset(S[:], 0.0)
                    for j in range(2):
                        for t_ in At[j]:
                            nc.vector.memset(t_[:], 0.0)

                # ================= layer 0 =================
                reset_states()
                emit_xg0_chunk(0)
                for k in range(NCH):
                    if k + 1 < NCH:
                        emit_xg0_chunk(k + 1)
                    for tr in range(CHT):
                        emit_step("L0", k * CHT + tr, T, Whh0t, True)

                # ================= layer 1 =================
                reset_states()
                emit_xg1_chunk(0)
                for k in range(NCH):
                    if k + 1 < NCH:
                        emit_xg1_chunk(k + 1)
                    for tr in range(CHT):
                        emit_step("L1", k * CHT + tr, T, Whh1t, False)

            # ================= U phase =================
            with tc.tile_pool(name="uw", bufs=2) as uw, \
                 tc.tile_pool(name="ups", bufs=2, space="PSUM") as ups:
                rowsets = [(0, 128), (128, 256), (256, 384), (384, 401)]
                for k in range(NUC):
                    c0 = k * 128
                    et = []
                    for (r0, r1) in rowsets:
                        t_ = uw.tile([r1 - r0, 128], BF16, tag=f"E{r0}", name=f"E{r0}")
                        nc.sync.dma_start(out=t_[:], in_=h1T[r0:r1, c0:c0 + 128])
                        et.append(t_)
                    psu = ups.tile([128, G4], F32, space="PSUM", tag="psu", name="psu")
                    for kc in range(4):
                        for (n0, n1) in ((0, 512), (512, G4)):
                            nc.tensor.matmul(psu[:, n0:n1], et[kc][:],
                                             WUt[kc][:, n0:n1],
                                             start=(kc == 0), stop=(kc == 3))
                    uo = uw.tile([128, G4], F32, tag="uo", name="uo")
                    nc.vector.tensor_copy(uo[:], psu[:])
                    nc.sync.dma_start(out=U0[c0:c0 + 128, :], in_=uo[:, 0:2 * H])
                    nc.sync.dma_start(out=U1[c0:c0 + 128, :], in_=uo[:, 2 * H:G4])

            # ================= final gather + MLP =================
            with tc.tile_pool(name="fw", bufs=2) as fw, \
                 tc.tile_pool(name="fc", bufs=1) as fc, \
                 tc.tile_pool(name="fps", bufs=2, space="PSUM") as fps:
                ui0 = fc.tile([128, NPT], I32)
                ui1 = fc.tile([128, NPT], I32)
                um0 = fc.tile([128, NPT], F32)
                um1 = fc.tile([128, NPT], F32)
                nc.sync.dma_start(out=ui0[:], in_=uidx0[:])
                nc.sync.dma_start(out=ui1[:], in_=uidx1[:])
                nc.sync.dma_start(out=um0[:], in_=umask0[:])
                nc.sync.dma_start(out=um1[:], in_=umask1[:])
                bwt = fc.tile([128, 2 * H], F32, name="bwt")
                nc.sync.dma_start(out=bwt[:], in_=bw1m[:])
                hm = [fc.tile([128, 512], BF16, tag=f"hm{i}", name=f"hm{i}") for i in range(2)]
                for t_ in hm:
                    nc.vector.memset(t_[:], 0.0)
                    nc.vector.memset(t_[:, 511:512], 1.0)
                for j in range(NPT):
                    par = j % 2
                    g0 = fw.tile([128, 2 * H], F32, tag="g0", name="g0")
                    g1 = fw.tile([128, 2 * H], F32, tag="g1", name="g1")
                    nc.gpsimd.indirect_dma_start(
                        out=g0[:], out_offset=None, in_=U0[:],
                        in_offset=bass.IndirectOffsetOnAxis(ap=ui0[:, j:j + 1], axis=0))
                    nc.gpsimd.indirect_dma_start(
                        out=g1[:], out_offset=None, in_=U1[:],
                        in_offset=bass.IndirectOffsetOnAxis(ap=ui1[:, j:j + 1], axis=0))
                    g1m = fw.tile([128, 2 * H], F32, tag="g1m", name="g1m")
                    nc.vector.scalar_tensor_tensor(g1m[:], g1[:], um1[:, j:j + 1],
                                                   bwt[:], ALU.mult, ALU.add)
                    ssum = fw.tile([128, 2 * H], F32, tag="ssum", name="ssum")
                    nc.vector.scalar_tensor_tensor(ssum[:], g0[:], um0[:, j:j + 1],
                                                   g1m[:], ALU.mult, ALU.add)
                    nc.scalar.activation(hm[par][:, 0:2 * H], ssum[:], AF.Tanh)
                    hmT = []
                    for i in range(4):
                        t_ = fw.tile([128, 128], BF16, tag=f"hmT{i}", name=f"hmT{i}")
                        nc.sync.dma_start_transpose(
                            out=t_[:], in_=hm[par][:, i * 128:(i + 1) * 128])
                        hmT.append(t_)
                    psl = fps.tile([128, 4], F32, space="PSUM", tag="psl", name="psl")
                    for i in range(4):
                        nc.tensor.matmul(psl[:], hmT[i][:], W2t[i][:],
                                         start=(i == 0), stop=(i == 3))
                    ex = fw.tile([128, 4], F32, tag="ex", name="ex")
                    nc.scalar.activation(ex[:], psl[:], AF.Exp)
                    sm = fw.tile([128, 1], F32, tag="sm", name="sm")
                    nc.vector.reduce_sum(sm[:], ex[:], axis=mybir.AxisListType.X)
                    rc = fw.tile([128, 1], F32, tag="rc", name="rc")
                    nc.vector.reciprocal(rc[:], sm[:])
                    ot = fw.tile([128, 4], F32, tag="ot", name="ot")
                    nc.vector.tensor_scalar_mul(ot[:], ex[:], rc[:, 0:1])
                    nc.sync.dma_start(out=OUT[j * 128:(j + 1) * 128, :], in_=ot[:])
    nc.compile()
    return nc


# ---------------------------------------------------------------------------
# host-side preparation
# ---------------------------------------------------------------------------

def _perm_gates(w):
    """torch gate order (i,f,g,o) -> (f,i,g,o) along axis 0 (4H rows)."""
    Hq = w.shape[0] // 4
    i, f, g, o = (w[0:Hq], w[Hq:2 * Hq], w[2 * Hq:3 * Hq], w[3 * Hq:4 * Hq])
    return np.concatenate([f, i, g, o], axis=0)


def _bd_stream(wT_f, wT_b, bias_f, bias_b, kpad):
    """Block-diag stream [2*kpad, G4]: rows [0:K] = wT_f, [kpad-1] = bias_f, ..."""
    K = wT_f.shape[0]
    out = np.zeros((2 * kpad, wT_f.shape[1]), np.float32)
    out[0:K] = wT_f
    out[kpad - 1] = bias_f
    out[kpad:kpad + K] = wT_b
    out[2 * kpad - 1] = bias_b
    return out


def prepare_inputs(inputs, T, n_cores):
    bf = ml_dtypes.bfloat16
    C_ = np.asarray(inputs["confs"]).shape[1]
    emb = np.asarray(inputs["emb"], np.float32)
    tokens = np.asarray(inputs["tokens"])
    confs = np.asarray(inputs["confs"])

    p = {}
    p["emb"] = emb.astype(bf)

    Wih0f = _perm_gates(np.asarray(inputs["Wih0f"], np.float32))
    Wih0b = _perm_gates(np.asarray(inputs["Wih0b"], np.float32))
    b0f = _perm_gates(np.asarray(inputs["b0f"], np.float32))
    b0b = _perm_gates(np.asarray(inputs["b0b"], np.float32))
    Whh0f = _perm_gates(np.asarray(inputs["Whh0f"], np.float32))
    Whh0b = _perm_gates(np.asarray(inputs["Whh0b"], np.float32))
    Wih1f = _perm_gates(np.asarray(inputs["Wih1f"], np.float32))
    Wih1b = _perm_gates(np.asarray(inputs["Wih1b"], np.float32))
    b1f = _perm_gates(np.asarray(inputs["b1f"], np.float32))
    b1b = _perm_gates(np.asarray(inputs["b1b"], np.float32))
    Whh1f = _perm_gates(np.asarray(inputs["Whh1f"], np.float32))
    Whh1b = _perm_gates(np.asarray(inputs["Whh1b"], np.float32))
    w1 = np.asarray(inputs["w1"], np.float32)
    bw1 = np.asarray(inputs["bw1"], np.float32)
    w2 = np.asarray(inputs["w2"], np.float32)
    bw2 = np.asarray(inputs["bw2"], np.float32)

    p["W0s"] = _bd_stream(Wih0f.T, Wih0b.T, b0f, b0b, EP).astype(bf)
    p["Whh0s"] = _bd_stream(Whh0f.T, Whh0b.T, 0 * b0f, 0 * b0b, HP).astype(bf)
    p["W1sf"] = np.concatenate([Wih1f.T, b1f[None, :]], 0).astype(bf)
    p["W1sb"] = np.concatenate([Wih1b.T, b1b[None, :]], 0).astype(bf)
    p["Whh1s"] = _bd_stream(Whh1f.T, Whh1b.T, 0 * b1f, 0 * b1b, HP).astype(bf)
    wu = np.concatenate([np.concatenate([w1[:, 0:2 * H].T, w1[:, 2 * H:].T], 1),
                         np.zeros((1, 2 * G4 // 2), np.float32)], 0)
    p["WU"] = wu.astype(bf)
    p["bw1m"] = np.tile(bw1[None, :], (128, 1)).astype(np.float32)
    w2p = np.zeros((512, 4), np.float32)
    w2p[0:2 * H] = w2.T
    w2p[511] = bw2
    p["W2s"] = w2p.astype(bf)

    NCH = T // CHT
    NP = BL * C_
    NPT = (NP + 127) // 128

    in_maps = []
    for c in range(n_cores):
        m = dict(p)
        bs = tokens[c * BL:(c + 1) * BL, 0:T]          # [BL, T]
        tf = np.zeros((CHT * BL, NCH), np.int32)
        tb = np.zeros((CHT * BL, NCH), np.int32)
        for k in range(NCH):
            for tr in range(CHT):
                tf[tr * BL:(tr + 1) * BL, k] = bs[:, k * CHT + tr]
                tb[tr * BL:(tr + 1) * BL, k] = bs[:, T - 1 - (k * CHT + tr)]
        m["tokf"] = tf
        m["tokb"] = tb
        cf = confs[c * BL:(c + 1) * BL]                 # [BL, C, 2]
        t0 = cf[:, :, 0].reshape(-1)                    # row-major (b, ci)
        t1 = cf[:, :, 1].reshape(-1)
        bidx = np.repeat(np.arange(BL), C_)
        ui0 = np.clip(t0, 0, T - 1) * BL + bidx
        ui1 = np.clip(t1, 0, T - 1) * BL + bidx
        um0 = (t0 >= 0).astype(np.float32)
        um1 = (t1 >= 0).astype(np.float32)

        def tile128(a, dt):
            o = np.zeros((NPT * 128,), dt)
            o[:a.shape[0]] = a
            return o.reshape(NPT, 128).T.copy()
        m["uidx0"] = tile128(ui0.astype(np.int32), np.int32)
        m["uidx1"] = tile128(ui1.astype(np.int32), np.int32)
        m["umask0"] = tile128(um0, np.float32)
        m["umask1"] = tile128(um1, np.float32)
        in_maps.append(m)
    return in_maps


_CACHE = {}


def _get_prog(T, n_cores, NPT):
    key = (T, n_cores, NPT)
    if key not in _CACHE:
        _CACHE[key] = build(T, n_cores, NPT)
    return _CACHE[key]


def kernel(**inputs):
    T = inputs["tokens"].shape[1]
    C_ = inputs["confs"].shape[1]
    n_cores = NCORE
    NP = BL * C_
    NPT = (NP + 127) // 128
    nc = _get_prog(T, n_cores, NPT)
    in_maps = prepare_inputs(inputs, T, n_cores)
    res = run_bass_kernel_spmd(nc, in_maps, list(range(n_cores)))
    outs = []
    for c in range(n_cores):
        o = res.results[c]["OUT"][:NP]          # [BL*C, 4] rows (b, ci)
        outs.append(o)
    return np.concatenate(outs, axis=0).astype(np.float32)



# revision 10
# speedup vs baseline: 2.4527x; 2.4527x over previous
"""Trainium2 Bass kernel for nn_BiLSTMNet (2-layer BiLSTM + pair-gather MLP).

Strategy: data-parallel across 8 cores (16 sentences each).  The whole
recurrence runs in TRANSPOSED layout: gates/h/c live as [feature-partitions,
(t, b) free columns], so the per-step recurrent matmuls stream only N=16
columns, the gate nonlinearities are 128-partition-wide with tiny free dims,
and h is written directly into a persistent SBUF mega-tile (no DRAM h traffic,
no per-step transposes).  The f/b directions are independent chains that
pipeline across engines.  Layer-1 input projections read h0 straight from
SBUF.  U = h1 @ w1-parts is computed per 128-slot chunk (h1 mega-tile slices
are ready-made lhsT), stored to DRAM, and the conf-pair gather + MLP runs as
row gathers + STT + tanh + PE-transpose + tiny matmul + softmax.

Gate row order is permuted host-side from torch (i,f,g,o) to (f,i,o,g) so one
sigmoid covers q-blocks 0..5 and one tanh covers q-blocks 6..7.
"""
import sys
sys.path.insert(0, "/opt/trn_rl_repo")
import numpy as np
import ml_dtypes

import concourse.bass as bass
import concourse.tile as tile
from concourse import mybir, bacc
from concourse.bass_utils import run_bass_kernel_spmd
from concourse.masks import make_identity

BF16 = mybir.dt.bfloat16
F32 = mybir.dt.float32
I32 = mybir.dt.int32
AF = mybir.ActivationFunctionType
ALU = mybir.AluOpType

DEBUG_STEPS = False
V, E, H, B, C = 32000, 200, 200, 128, 256
T_FULL = 512
BL = 16            # sentences per core
NCORE = 8
CHT = 4            # timesteps per xg chunk
HH = 100           # half of H (q-block height)
NQ = 8             # q-blocks per direction (f0,f1,i0,i1,o0,o1,g0,g1)


def build(T, n_cores, NPT, debug_dump=False):
    NCH = T // CHT
    NSLOT = T * BL
    NUC = NSLOT // 128
    HCOLS = 4 * NSLOT          # h mega-tile cols: (hh, d, t, b)

    nc = bacc.Bacc("TRN2", target_bir_lowering=False, debug=False,
                   enable_asserts=True, num_devices=n_cores)

    def din(name, shape, dt):
        return nc.dram_tensor(name, shape, dt, kind="ExternalInput").ap()

    def dout(name, shape, dt):
        return nc.dram_tensor(name, shape, dt, kind="ExternalOutput").ap()

    emb = din("emb", [V, E], BF16)
    W0 = din("W0", [128, 3200], BF16)     # L0 Wih lhsT chunks (d,q,e)
    Wr0 = din("Wr0", [100, 3200], BF16)   # L0 Whh lhsT chunks (d,q,hh)
    W1 = din("W1", [101, 6400], BF16)     # L1 Wih lhsT chunks (d,q,kb)
    Wr1 = din("Wr1", [100, 3200], BF16)
    WU = din("WU", [100, 3200], BF16)     # U rhs chunks (kb)
    W2s = din("W2s", [128, 16], BF16)     # final lhs-rhs chunks
    onesrow = din("onesrow", [1, HCOLS], BF16)
    tokf = din("tokf", [CHT * BL, NCH], I32)
    tokb = din("tokb", [CHT * BL, NCH], I32)
    uidx0 = din("uidx0", [128, NPT], I32)
    uidx1 = din("uidx1", [128, NPT], I32)
    umask0 = din("umask0", [128, NPT], F32)
    umask1 = din("umask1", [128, NPT], F32)
    bw1m = din("bw1m", [128, 2 * H], F32)

    OUT = dout("OUT", [NPT * 128, 4], F32)

    U0d = nc.dram_tensor("U0d", [NSLOT, 2 * H], F32).ap()
    U1d = nc.dram_tensor("U1d", [NSLOT, 2 * H], F32).ap()

    def hcol(hh, d, t):
        return hh * (2 * NSLOT) + d * NSLOT + t * BL

    with tile.TileContext(nc) as tc:
        with tc.tile_pool(name="const", bufs=1) as cp, \
             tc.tile_pool(name="mega", bufs=1) as mp, \
             tc.tile_pool(name="state", bufs=1) as sp:

            def load(src, shape, dt):
                t_ = cp.tile(shape, dt, tag=f"w{src.name}", name=f"w{src.name}")
                nc.sync.dma_start(out=t_[:], in_=src[:])
                return t_

            W0t = load(W0.tensor.ap(), [128, 3200], BF16)
            Wr0t = load(Wr0.tensor.ap(), [100, 3200], BF16)
            W1t = load(W1.tensor.ap(), [101, 6400], BF16)
            Wr1t = load(Wr1.tensor.ap(), [100, 3200], BF16)
            WUt = load(WU.tensor.ap(), [100, 3200], BF16)
            W2t = load(W2s.tensor.ap(), [128, 16], BF16)
            tokf_t = load(tokf.tensor.ap(), [CHT * BL, NCH], I32)
            tokb_t = load(tokb.tensor.ap(), [CHT * BL, NCH], I32)

            h0 = mp.tile([101, HCOLS], BF16, name="h0")
            h1 = mp.tile([101, HCOLS], BF16, name="h1")
            hc2 = HCOLS // 2
            nc.sync.dma_start(out=h0[100:101, 0:hc2], in_=onesrow[:, 0:hc2])
            nc.sync.dma_start(out=h0[100:101, hc2:HCOLS],
                              in_=onesrow[:, hc2:HCOLS])

            # x gather tiles (per parity, per dir): cols 200:255 zero, 255 one
            gx = [[sp.tile([CHT * BL, 256], BF16, tag=f"gx{d}{i}", name=f"gx{d}{i}")
                   for i in range(2)] for d in range(2)]
            for d in range(2):
                for i in range(2):
                    nc.vector.memset(gx[d][i][:], 0.0)
                    nc.vector.memset(gx[d][i][:, 255:256], 1.0)
            # xT tiles [128, 64] per (d, e, parity)
            xT = [[[sp.tile([128, CHT * BL], BF16, tag=f"xT{d}{e}{i}",
                            name=f"xT{d}{e}{i}") for i in range(2)]
                   for e in range(2)] for d in range(2)]
            # cell state per dir [100, 2*BL] (cols hh*BL + b)
            c_t = [sp.tile([HH, 2 * BL], F32, tag=f"c{d}", name=f"c{d}")
                   for d in range(2)]
            ident = sp.tile([128, 128], BF16, name="ident")
            make_identity(nc, ident[:])

            toks = [tokf_t, tokb_t]

            with tc.tile_pool(name="work", bufs=2) as wp, \
                 tc.tile_pool(name="xps", bufs=2, space="PSUM") as xps:

                xg_tiles = {}

                def emit_prefetch(k):
                    par = k % 2
                    for d in range(2):
                        g = gx[d][par]
                        nc.gpsimd.indirect_dma_start(
                            out=g[:, 0:E], out_offset=None, in_=emb[:],
                            in_offset=bass.IndirectOffsetOnAxis(
                                ap=toks[d][:, k:k + 1], axis=0))
                        for e in range(2):
                            nc.sync.dma_start_transpose(
                                out=xT[d][e][par][:],
                                in_=g[:, e * 128:(e + 1) * 128])

                def alloc_P(layer, k):
                    Pd = [xps.tile([HH, 512], F32, space="PSUM", tag=f"P{d}",
                                   name=f"P{d}", padded_shape=[HH, 512])
                          for d in range(2)]
                    xg_tiles[(layer, k)] = Pd
                    return Pd

                def emit_xg0(k, qs):
                    par = k % 2
                    Pd = xg_tiles[(0, k)]
                    for d in range(2):
                        for q in qs:
                            m = d * 16 + q * 2
                            for e in range(2):
                                # one start=True per PSUM bank per round: it
                                # marks the whole 2KB bank pending-zero
                                nc.tensor.matmul(
                                    Pd[d][:, q * 64:(q + 1) * 64],
                                    W0t[:, (m + e) * 100:(m + e + 1) * 100],
                                    xT[d][e][par][:],
                                    start=(q == 0 and e == 0), stop=False,
                                    skip_group_check=True)

                def emit_xg1(k, qs):
                    Pd = xg_tiles[(1, k)]
                    h0v = h0[:].rearrange("p (hh d t b) -> p hh d t b",
                                          hh=2, d=2, b=BL)
                    for d in range(2):
                        for q in qs:
                            m = d * 32 + q * 4
                            for kb in range(4):
                                hh, dp = kb % 2, kb // 2
                                K = 101 if kb == 3 else 100
                                if d == 0:
                                    rhs = h0v[0:K, hh, dp,
                                              k * CHT:(k + 1) * CHT, :]
                                else:
                                    t0 = T - 1 - k * CHT
                                    t_sl = (slice(t0, None, -1) if t0 - CHT < 0
                                            else slice(t0, t0 - CHT, -1))
                                    rhs = h0v[0:K, hh, dp, t_sl, :]
                                nc.tensor.matmul(
                                    Pd[d][:, q * 64:(q + 1) * 64],
                                    W1t[0:K, (m + kb) * 100:(m + kb + 1) * 100],
                                    rhs,
                                    start=(q == 0 and kb == 0), stop=False,
                                    skip_group_check=True)

                def emit_step(layer, k, tr, Wrt, hout):
                    p = k * CHT + tr
                    Pd = xg_tiles[(layer, k)]
                    # recurrent matmuls (skip at p=0: h_init = 0)
                    if p > 0:
                        for d in range(2):
                            tp = p - 1 if d == 0 else T - p
                            for q in range(NQ):
                                m = d * 16 + q * 2
                                for hh in range(2):
                                    co = hcol(hh, d, tp)
                                    nc.tensor.matmul(
                                        Pd[d][:, q * 64 + tr * BL:
                                              q * 64 + (tr + 1) * BL],
                                        Wrt[:, (m + hh) * 100:(m + hh + 1) * 100],
                                        hout[0:100, co:co + BL],
                                        start=False, stop=(hh == 1),
                                        skip_group_check=True)
                    sigs = []
                    for d in range(2):
                        P4 = Pd[d][:].rearrange("p (q t b) -> p q t b",
                                                q=NQ, b=BL)
                        sg = wp.tile([HH, 8 * BL], F32, tag=f"sig{d}",
                                     name=f"sig{d}")
                        s3 = sg[:].rearrange("p (q b) -> p q b", b=BL)
                        nc.scalar.activation(s3[:, 0:6, :],
                                             P4[:, 0:6, tr:tr + 1, :],
                                             AF.Sigmoid)
                        nc.scalar.activation(s3[:, 6:8, :],
                                             P4[:, 6:8, tr:tr + 1, :],
                                             AF.Tanh)
                        sigs.append(sg)
                    tcs = []
                    for d in range(2):
                        sg = sigs[d]
                        pr2 = wp.tile([HH, 2 * BL], F32, tag=f"pr2{d}",
                                      name=f"pr2{d}")
                        nc.vector.tensor_mul(pr2[:], sg[:, 2 * BL:4 * BL],
                                             sg[:, 6 * BL:8 * BL])
                        pr1 = wp.tile([HH, 2 * BL], F32, tag=f"pr1{d}",
                                      name=f"pr1{d}")
                        nc.vector.tensor_mul(pr1[:], sg[:, 0:2 * BL],
                                             c_t[d][:])
                        nc.vector.tensor_add(c_t[d][:], pr1[:], pr2[:])
                        tc_ = wp.tile([HH, 2 * BL], F32, tag=f"tc{d}",
                                      name=f"tc{d}")
                        nc.scalar.activation(tc_[:], c_t[d][:], AF.Tanh)
                        tcs.append(tc_)
                    for d in range(2):
                        t_sent = p if d == 0 else T - 1 - p
                        hv = hout[0:100, :].rearrange("p (hh x) -> p hh x",
                                                      hh=2)
                        co = d * NSLOT + t_sent * BL
                        nc.vector.tensor_mul(hv[:, :, co:co + BL],
                                             sigs[d][:, 4 * BL:6 * BL],
                                             tcs[d][:])
                    if DEBUG_STEPS and layer == 0 and p < 3:
                        for d in range(2):
                            ds = nc.dram_tensor(f"dbg_s{p}_{d}", [HH, 8 * BL],
                                                F32, kind="ExternalOutput").ap()
                            nc.sync.dma_start(out=ds[:], in_=sigs[d][:])
                            dc = nc.dram_tensor(f"dbg_c{p}_{d}", [HH, 2 * BL],
                                                F32, kind="ExternalOutput").ap()
                            nc.sync.dma_start(out=dc[:], in_=c_t[d][:])

                QGROUPS = [(0, 1), (2, 3), (4, 5), (6, 7)]

                # ================= layer 0 =================
                for d in range(2):
                    nc.vector.memset(c_t[d][:], 0.0)
                emit_prefetch(0)
                emit_prefetch(1)
                alloc_P(0, 0)
                emit_xg0(0, range(NQ))
                for k in range(NCH):
                    if k + 2 < NCH:
                        emit_prefetch(k + 2)
                    if k + 1 < NCH:
                        alloc_P(0, k + 1)
                    for tr in range(CHT):
                        if k + 1 < NCH:
                            emit_xg0(k + 1, QGROUPS[tr])
                        emit_step(0, k, tr, Wr0t, h0)
                    xg_tiles.pop((0, k))

                # ================= layer 1 =================
                for d in range(2):
                    nc.vector.memset(c_t[d][:], 0.0)
                alloc_P(1, 0)
                emit_xg1(0, range(NQ))
                for k in range(NCH):
                    if k + 1 < NCH:
                        alloc_P(1, k + 1)
                    for tr in range(CHT):
                        if k + 1 < NCH:
                            emit_xg1(k + 1, QGROUPS[tr])
                        emit_step(1, k, tr, Wr1t, h1)
                    xg_tiles.pop((1, k))

            if debug_dump:
                h0dbg = nc.dram_tensor("h0dbg", [101, HCOLS], BF16,
                                       kind="ExternalOutput").ap()
                h1dbg = nc.dram_tensor("h1dbg", [101, HCOLS], BF16,
                                       kind="ExternalOutput").ap()
                nc.sync.dma_start(out=h0dbg[:, 0:HCOLS // 2],
                                  in_=h0[:, 0:HCOLS // 2])
                nc.sync.dma_start(out=h0dbg[:, HCOLS // 2:],
                                  in_=h0[:, HCOLS // 2:])
                nc.sync.dma_start(out=h1dbg[0:100, 0:HCOLS // 2],
                                  in_=h1[0:100, 0:HCOLS // 2])
                nc.sync.dma_start(out=h1dbg[0:100, HCOLS // 2:],
                                  in_=h1[0:100, HCOLS // 2:])

            # ================= U phase =================
            with tc.tile_pool(name="uw", bufs=3) as uw, \
                 tc.tile_pool(name="ups", bufs=2, space="PSUM") as ups:
                for sc in range(NUC):
                    ps1 = ups.tile([128, 512], F32, space="PSUM", tag="ups1",
                                   name="ups1", padded_shape=[128, 512])
                    ps2 = ups.tile([128, 288], F32, space="PSUM", tag="ups2",
                                   name="ups2", padded_shape=[128, 512])
                    for kb in range(4):
                        hh, dd = kb % 2, kb // 2
                        lhsT = h1[0:100, hcol(hh, dd, 0) + sc * 128:
                                  hcol(hh, dd, 0) + (sc + 1) * 128]
                        nc.tensor.matmul(ps1[:], lhsT,
                                         WUt[:, kb * 800:kb * 800 + 512],
                                         start=(kb == 0), stop=(kb == 3))
                        nc.tensor.matmul(ps2[:], lhsT,
                                         WUt[:, kb * 800 + 512:(kb + 1) * 800],
                                         start=(kb == 0), stop=(kb == 3))
                    uo = uw.tile([128, 800], F32, tag="uo", name="uo")
                    nc.vector.tensor_copy(uo[:, 0:512], ps1[:])
                    nc.vector.tensor_copy(uo[:, 512:800], ps2[:])
                    nc.sync.dma_start(out=U0d[sc * 128:(sc + 1) * 128, :],
                                      in_=uo[:, 0:2 * H])
                    nc.sync.dma_start(out=U1d[sc * 128:(sc + 1) * 128, :],
                                      in_=uo[:, 2 * H:4 * H])

            # ================= gather + MLP =================
            with tc.tile_pool(name="fw", bufs=3) as fw, \
                 tc.tile_pool(name="fc", bufs=1) as fc, \
                 tc.tile_pool(name="fpsT", bufs=1, space="PSUM") as fpsT, \
                 tc.tile_pool(name="fps", bufs=2, space="PSUM") as fps:
                ui0 = fc.tile([128, NPT], I32, name="ui0")
                ui1 = fc.tile([128, NPT], I32, name="ui1")
                um0 = fc.tile([128, NPT], F32, name="um0")
                um1 = fc.tile([128, NPT], F32, name="um1")
                nc.sync.dma_start(out=ui0[:], in_=uidx0[:])
                nc.sync.dma_start(out=ui1[:], in_=uidx1[:])
                nc.sync.dma_start(out=um0[:], in_=umask0[:])
                nc.sync.dma_start(out=um1[:], in_=umask1[:])
                bwt = fc.tile([128, 2 * H], F32, name="bwt")
                nc.sync.dma_start(out=bwt[:], in_=bw1m[:])
                hm = [fc.tile([128, 512], BF16, tag=f"hm{i}", name=f"hm{i}")
                      for i in range(2)]
                for t_ in hm:
                    nc.vector.memset(t_[:], 0.0)
                    nc.vector.memset(t_[:, 511:512], 1.0)
                for j in range(NPT):
                    par = j % 2
                    g0 = fw.tile([128, 2 * H], F32, tag="g0", name="g0")
                    g1 = fw.tile([128, 2 * H], F32, tag="g1", name="g1")
                    nc.gpsimd.indirect_dma_start(
                        out=g0[:], out_offset=None, in_=U0d[:],
                        in_offset=bass.IndirectOffsetOnAxis(
                            ap=ui0[:, j:j + 1], axis=0))
                    nc.gpsimd.indirect_dma_start(
                        out=g1[:], out_offset=None, in_=U1d[:],
                        in_offset=bass.IndirectOffsetOnAxis(
                            ap=ui1[:, j:j + 1], axis=0))
                    g1m = fw.tile([128, 2 * H], F32, tag="g1m", name="g1m")
                    nc.vector.scalar_tensor_tensor(g1m[:], g1[:],
                                                   um1[:, j:j + 1], bwt[:],
                                                   ALU.mult, ALU.add)
                    ssum = fw.tile([128, 2 * H], F32, tag="ssum", name="ssum")
                    nc.vector.scalar_tensor_tensor(ssum[:], g0[:],
                                                   um0[:, j:j + 1], g1m[:],
                                                   ALU.mult, ALU.add)
                    nc.scalar.activation(hm[par][:, 0:2 * H], ssum[:], AF.Tanh)
                    psT = []
                    for i in range(4):
                        pt = fpsT.tile([128, 128], BF16, space="PSUM",
                                       tag=f"pT{i}", name=f"pT{i}",
                                       padded_shape=[128, 1024])
                        nc.tensor.transpose(pt[:],
                                            hm[par][:, i * 128:(i + 1) * 128],
                                            ident[:])
                        psT.append(pt)
                    hT = []
                    for i in range(4):
                        ht_ = fw.tile([128, 128], BF16, tag=f"hT{i}",
                                      name=f"hT{i}")
                        nc.vector.tensor_copy(ht_[:], psT[i][:])
                        hT.append(ht_)
                    psl = fps.tile([128, 4], F32, space="PSUM", tag="psl",
                                   name="psl", padded_shape=[128, 512])
                    for i in range(4):
                        nc.tensor.matmul(psl[:], hT[i][:],
                                         W2t[:, i * 4:(i + 1) * 4],
                                         start=(i == 0), stop=(i == 3))
                    ex = fw.tile([128, 4], F32, tag="ex", name="ex")
                    nc.scalar.activation(ex[:], psl[:], AF.Exp)
                    sm = fw.tile([128, 1], F32, tag="sm", name="sm")
                    nc.vector.reduce_sum(sm[:], ex[:], axis=mybir.AxisListType.X)
                    rc = fw.tile([128, 1], F32, tag="rc", name="rc")
                    nc.vector.reciprocal(rc[:], sm[:])
                    ot = fw.tile([128, 4], F32, tag="ot", name="ot")
                    nc.vector.tensor_scalar_mul(ot[:], ex[:], rc[:, 0:1])
                    nc.sync.dma_start(out=OUT[j * 128:(j + 1) * 128, :],
                                      in_=ot[:])
    nc.compile()
    return nc


# ---------------------------------------------------------------------------
# host-side preparation
# ---------------------------------------------------------------------------

def _perm_rows(w):
    """torch gate order (i,f,g,o) -> (f,i,o,g) along axis 0."""
    i, f, g, o = np.split(w, 4, axis=0)
    return np.concatenate([f, i, o, g], axis=0)


def prepare_inputs(inputs, T, n_cores):
    bf = ml_dtypes.bfloat16
    C_ = np.asarray(inputs["confs"]).shape[1]
    NSLOT = T * BL
    NCH = T // CHT
    emb = np.asarray(inputs["emb"], np.float32)
    tokens = np.asarray(inputs["tokens"])
    confs = np.asarray(inputs["confs"])

    p = {}
    p["emb"] = emb.astype(bf)

    def wihT(name):
        return _perm_rows(np.asarray(inputs[name], np.float32)).T.copy()

    def bia(name):
        return _perm_rows(np.asarray(inputs[name], np.float32)[:, None])[:, 0]

    # --- L0 Wih lhsT chunks [128, 3200]: m = d*16 + q*2 + e
    W0p = np.zeros((128, 3200), np.float32)
    for d, (wn, bn) in enumerate([("Wih0f", "b0f"), ("Wih0b", "b0b")]):
        wT, bb = wihT(wn), bia(bn)          # [200, 800], [800]
        for q in range(NQ):
            cb = wT[:, q * 100:(q + 1) * 100]
            m0 = (d * 16 + q * 2) * 100
            W0p[0:128, m0:m0 + 100] = cb[0:128]
            W0p[0:72, m0 + 100:m0 + 200] = cb[128:200]
            W0p[127, m0 + 100:m0 + 200] = bb[q * 100:(q + 1) * 100]
    p["W0"] = W0p.astype(bf)

    # --- L0 Whh lhsT chunks [100, 3200]: m = d*16 + q*2 + hh
    def rec_pack(wf, wb):
        out = np.zeros((100, 3200), np.float32)
        for d, wn in enumerate([wf, wb]):
            wT = wihT(wn)                    # [200, 800]
            for q in range(NQ):
                cb = wT[:, q * 100:(q + 1) * 100]
                m0 = (d * 16 + q * 2) * 100
                out[:, m0:m0 + 100] = cb[0:100]
                out[:, m0 + 100:m0 + 200] = cb[100:200]
        return out
    p["Wr0"] = rec_pack("Whh0f", "Whh0b").astype(bf)
    p["Wr1"] = rec_pack("Whh1f", "Whh1b").astype(bf)

    # --- L1 Wih lhsT chunks [101, 6400]: m = d*32 + q*4 + kb
    W1p = np.zeros((101, 6400), np.float32)
    for d, (wn, bn) in enumerate([("Wih1f", "b1f"), ("Wih1b", "b1b")]):
        wT, bb = wihT(wn), bia(bn)          # [400, 800], [800]
        for q in range(NQ):
            cb = wT[:, q * 100:(q + 1) * 100]
            for kb in range(4):
                m0 = (d * 32 + q * 4 + kb) * 100
                W1p[0:100, m0:m0 + 100] = cb[kb * 100:(kb + 1) * 100]
            W1p[100, (d * 32 + q * 4 + 3) * 100:
                 (d * 32 + q * 4 + 4) * 100] = bb[q * 100:(q + 1) * 100]
    p["W1"] = W1p.astype(bf)

    # --- U rhs chunks [100, 3200]: kb blocks of w1rhs [400, 800]
    w1 = np.asarray(inputs["w1"], np.float32)
    w1rhs = np.concatenate([w1[:, 0:400].T, w1[:, 400:800].T], axis=1)
    WUp = np.zeros((100, 3200), np.float32)
    for kb in range(4):
        WUp[:, kb * 800:(kb + 1) * 800] = w1rhs[kb * 100:(kb + 1) * 100]
    p["WU"] = WUp.astype(bf)
    p["bw1m"] = np.tile(np.asarray(inputs["bw1"], np.float32)[None, :],
                        (128, 1)).astype(np.float32)

    w2 = np.asarray(inputs["w2"], np.float32)
    bw2 = np.asarray(inputs["bw2"], np.float32)
    w2p = np.zeros((512, 4), np.float32)
    w2p[0:400] = w2.T
    w2p[511] = bw2
    W2sp = np.zeros((128, 16), np.float32)
    for cgroup in range(4):
        W2sp[:, cgroup * 4:(cgroup + 1) * 4] = w2p[cgroup * 128:
                                                   (cgroup + 1) * 128]
    p["W2s"] = W2sp.astype(bf)

    p["onesrow"] = np.ones((1, 4 * NSLOT), np.float32).astype(bf)

    NP = BL * C_
    NPT = (NP + 127) // 128

    in_maps = []
    for cc in range(n_cores):
        m = dict(p)
        bs = tokens[cc * BL:(cc + 1) * BL, 0:T]          # [BL, T]
        tf = np.zeros((CHT * BL, NCH), np.int32)
        tb = np.zeros((CHT * BL, NCH), np.int32)
        for k in range(NCH):
            for tr in range(CHT):
                tf[tr * BL:(tr + 1) * BL, k] = bs[:, k * CHT + tr]
                tb[tr * BL:(tr + 1) * BL, k] = bs[:, T - 1 - (k * CHT + tr)]
        m["tokf"] = tf
        m["tokb"] = tb
        cf = confs[cc * BL:(cc + 1) * BL]                 # [BL, C, 2]
        t0 = cf[:, :, 0].reshape(-1)
        t1 = cf[:, :, 1].reshape(-1)
        bidx = np.repeat(np.arange(BL), C_)
        ui0 = np.clip(t0, 0, T - 1) * BL + bidx
        ui1 = np.clip(t1, 0, T - 1) * BL + bidx
        um0 = (t0 >= 0).astype(np.float32)
        um1 = (t1 >= 0).astype(np.float32)

        def tile128(a, dt):
            o = np.zeros((NPT * 128,), dt)
            o[:a.shape[0]] = a
            return o.reshape(NPT, 128).T.copy()
        m["uidx0"] = tile128(ui0.astype(np.int32), np.int32)
        m["uidx1"] = tile128(ui1.astype(np.int32), np.int32)
        m["umask0"] = tile128(um0, np.float32)
        m["umask1"] = tile128(um1, np.float32)
        in_maps.append(m)
    return in_maps


_CACHE = {}


def _get_prog(T, n_cores, NPT):
    key = (T, n_cores, NPT)
    if key not in _CACHE:
        _CACHE[key] = build(T, n_cores, NPT)
    return _CACHE[key]


def kernel(**inputs):
    T = inputs["tokens"].shape[1]
    C_ = inputs["confs"].shape[1]
    n_cores = NCORE
    NP = BL * C_
    NPT = (NP + 127) // 128
    nc = _get_prog(T, n_cores, NPT)
    in_maps = prepare_inputs(inputs, T, n_cores)
    res = run_bass_kernel_spmd(nc, in_maps, list(range(n_cores)))
    outs = []
    for cc in range(n_cores):
        o = res.results[cc]["OUT"][:NP]
        outs.append(o)
    return np.concatenate(outs, axis=0).astype(np.float32)
